# revision 40
# baseline (speedup 1.0000x reference)
"""Causal multi-head attention (B=2, S=2048, E=1024, H=16, D=64) on 8 trn2 NeuronCores.

Sharding: core c handles batch b = c // 4 and head group g = c % 4 (4 heads each).
Each core computes, for its batch and its 4 heads:
    q/k/v = x @ W[qkv][:, 256g:256g+256], causal attention, then the partial
    projection  out_heads @ Wp[256g:256g+256, :]  -> [2048, 1024].
Host gathers: out[b] = sum_g partial[b, g] + bp  (the "all-reduce" of the TP hint).

v2 design notes (vs the f32r baseline):
- All matmul operands are fp16 (x and W are converted to fp16 on the host, the
  partial y is returned fp16 and reduced in fp32 on the host). 10-bit mantissa
  matches f32r precision; halves all DMA traffic.
- xT ([E,S] layout) is produced by XBAR DMA-transpose straight from DRAM, so
  the PE never runs transposes and there is no x-natural staging.
- Every matmul in the program runs in the 64x128 row-tiled PE mode: each
  logical 128-deep contraction is split into two 64-row halves placed at
  tile_position (0,0)/(64,0).  Pairs of matmuls that target different PSUM
  banks are emitted with complementary halves so the PE executes them
  concurrently (measured on HW: same-timestamp row_grp h0/h64 pairs), which
  keeps per-slot cost equal to the untiled version while avoiding any
  tiling-mode switch (mode switches drain the array).
- The Act engine only runs exp (the serial ~60us resource); scores for both
  heads of a pair land in one [128,1024] PSUM tile so each j-tile needs one
  ACTIVATE.  Causal masking of diagonal tiles is a DVE multiply with a
  constant lower-triangle fp16 mask; QK/exp/PV all stream only [cm, 512) of
  each diagonal block.
- QKV for s-chunk c and attention for q-block c-1 are software-pipelined:
  chunk jobs (and later the output projection) are generators that yield
  ~512-cycle slices, drained as PE filler between attention tiles so the PE
  never idles while Act churns through exp.
"""

import os
import sys
import numpy as np

sys.path.insert(0, "/opt/trn_rl_repo")

import concourse.bass as bass
import concourse.bacc as bacc_mod
import concourse.mybir as mybir
import concourse.tile as tile
from concourse import library_config

F32 = mybir.dt.float32
F16 = mybir.dt.float16
P = 128
H = 64                # contraction half (row-tile height)

B = 2
S = 2048
E = 1024
NHEADS_TOTAL = 16
D = 64
N_CORES = 8
GROUPS = 4            # head groups (tensor parallel)
HD = NHEADS_TOTAL * D // GROUPS   # 256 head-dims per core
NH = HD // D          # heads per core (4)
NHP = HD // P         # head pairs (2)
NST = S // P          # s tiles (16)
NEC = E // P          # e chunks (8)
QB = 512              # q-block width
NQB = S // QB         # q blocks (4)
JPQ = QB // P         # j tiles per q block (4)
VW = D + 1            # 65: v columns + ones column

EXP = mybir.ActivationFunctionType.Exp

# debug toggles
# DMA-transpose is functionally fine but each issue instruction occupies the
# SP engine ~1.3us (descriptor gen) — 32 issues poison the whole schedule.
# PE transposes in fp16 cost 53ns/tile and ride in idle PE slots instead.
XT_DMA_TRANSPOSE = os.environ.get("KQ_XT_DMA", "0") == "1"
# half-split row tiling hangs on HW (same-bank access from two row groups);
# full-contraction matmuls cost the same cycles anyway.
SPLIT_HALVES = os.environ.get("KQ_SPLIT", "0") == "1"


def build_core_program(lower_isa=True):
    """One NeuronCore's program (SPMD: all 8 cores run this on different data)."""
    nc = bacc_mod.Bacc()
    x_d = nc.declare_dram_parameter("x", [S, E], F16, False)
    wq_d = nc.declare_dram_parameter("wq", [E, HD], F16, False)
    wk_d = nc.declare_dram_parameter("wk", [E, HD], F16, False)
    wv_d = nc.declare_dram_parameter("wv", [E, HD], F16, False)
    wp_d = nc.declare_dram_parameter("wp", [HD, E], F16, False)
    # host-built constants: identity (PE transposes), causal keep-mask
    # (tri[r,c] = c>=r), all-ones (v_ext denominator column + the
    # denominator-broadcast outer product). Building these on-device costs
    # ~4us of gpsimd iota on the critical path; a DMA is ~100ns.
    id_d = nc.declare_dram_parameter("cident", [P, P], F16, False)
    tri_d = nc.declare_dram_parameter("ctri", [P, P], F16, False)
    one_d = nc.declare_dram_parameter("cones", [P, NST * NH], F16, False)
    y_d = nc.declare_dram_parameter("y", [S, E], F16, True)

    with tile.TileContext(nc) as tc:
        # "standard" provides InstIota (make_identity) + InstTensorTensor
        # (diag-mask multiplies on the pool engine)
        nc.gpsimd.load_library(library_config.standard)
        from contextlib import ExitStack
        with ExitStack() as ctx:
            const = ctx.enter_context(tc.tile_pool(name="const", bufs=1))
            persist = ctx.enter_context(tc.tile_pool(name="persist", bufs=1))

            # ---------------- persistent SBUF tiles ----------------
            xT = [persist.tile([P, S], F16, tag=f"xT{ec}", name=f"xT{ec}")
                  for ec in range(NEC)]
            qT = [persist.tile([P, S], F16, tag=f"qT{hp}", name=f"qT{hp}")
                  for hp in range(NHP)]
            kT = [persist.tile([P, S], F16, tag=f"kT{hp}", name=f"kT{hp}")
                  for hp in range(NHP)]
            v_ext = persist.tile([P, NST * NH * VW], F16, tag="v_ext",
                                 name="v_ext")
            oT_all = [persist.tile([P, S], F16, tag=f"oT{hp}", name=f"oT{hp}")
                      for hp in range(NHP)]
            wq_sb = persist.tile([P, NEC * HD], F16, tag="wq", name="wq_sb")
            wk_sb = persist.tile([P, NEC * HD], F16, tag="wk", name="wk_sb")
            wv_sb = persist.tile([P, NEC * HD], F16, tag="wv", name="wv_sb")
            wp_sb = [persist.tile([P, E], F16, tag=f"wp{hp}", name=f"wp{hp}")
                     for hp in range(NHP)]
            xn_all = None
            if not XT_DMA_TRANSPOSE:
                # x natural staging for the PE transposes
                xn_all = persist.tile([P, NST * E], F16, tag="xn", name="xn_all")
            ident = const.tile([P, P], F16)
            tri = const.tile([P, P], F16)
            ones16 = const.tile([P, D], F16)

            # ---------------- DMA (all prefetched up front) ----------------
            # chunk-0 x first (it gates the whole pipeline), split across the
            # two HWDGE issuing engines (SP + Activation); then wq/wk (needed
            # with chunk 0), then the rest.
            def xdma_t(sc, ec):
                nc.sync.dma_start_transpose(
                    xT[ec][:, QB * sc:QB * (sc + 1)],
                    x_d[QB * sc:QB * (sc + 1), P * ec:P * (ec + 1)],
                )

            def xdma_n(k):  # one [128, E] row-tile of x into staging
                nc.sync.dma_start(
                    out=xn_all.rearrange("p (k e) -> p k e", e=E)[:, k:k + 1, :],
                    in_=x_d[P * k:P * (k + 1), :].rearrange(
                        "(k p) e -> p k e", p=P),
                )

            def wdma(wd, wsb):
                nc.sync.dma_start(
                    out=wsb.rearrange("p (c n) -> p c n", c=NEC),
                    in_=wd[:, :].rearrange("(c p) n -> p c n", p=P),
                )

            # ident gates the transposes, wq the first qk matmuls — first
            nc.sync.dma_start(out=ident, in_=id_d[:, :])
            wdma(wq_d, wq_sb)
            if XT_DMA_TRANSPOSE:
                for ec in range(NEC):
                    xdma_t(0, ec)
            else:
                for k in range(4):
                    xdma_n(k)
            nc.sync.dma_start(out=tri, in_=tri_d[:, :])
            nc.sync.dma_start(out=ones16, in_=one_d[:, 0:D])
            # denominator ones-column of every v slab, straight from DRAM
            nc.sync.dma_start(
                out=v_ext.rearrange("p (s c) -> p s c", c=VW)[:, :, D:VW],
                in_=one_d[:, :].rearrange("p (s o) -> p s o", o=1),
            )
            wdma(wk_d, wk_sb)
            if XT_DMA_TRANSPOSE:
                for sc in range(1, NQB):
                    for ec in range(NEC):
                        xdma_t(sc, ec)
            else:
                for k in range(4, 8):
                    xdma_n(k)
                wdma(wv_d, wv_sb)
                for k in range(8, NST):
                    xdma_n(k)
            if XT_DMA_TRANSPOSE:
                wdma(wv_d, wv_sb)
            for hp in range(NHP):
                nc.sync.dma_start(out=wp_sb[hp], in_=wp_d[P * hp:P * (hp + 1), :])

            # warm up the Act exp table during the DMA wait
            warm = const.tile([1, 1], F32)
            nc.scalar.activation(warm[0:1, 0:1], ones16[0:1, 0:1], EXP,
                                 scale=1.0)

            # ---------------- PSUM pools ----------------
            # 8 banks total: sT 2x[128,1024]f32 (4) + oT 2x[128,512]f32 (2)
            # + aux 2x[128,512]f32 (2).
            sT_ps = ctx.enter_context(
                tc.tile_pool(name="sT_ps", bufs=2, space="PSUM"))
            oT_ps = ctx.enter_context(
                tc.tile_pool(name="oT_ps", bufs=2, space="PSUM"))
            aux_ps = ctx.enter_context(
                tc.tile_pool(name="aux_ps", bufs=2, space="PSUM"))
            pT_pool = ctx.enter_context(tc.tile_pool(name="pT", bufs=4))
            y_pool = ctx.enter_context(tc.tile_pool(name="ysb", bufs=2))
            o_pool = ctx.enter_context(tc.tile_pool(name="osb", bufs=4))
            rb_pool = ctx.enter_context(tc.tile_pool(name="rbp", bufs=4))

            # ---------------- filler job generators ----------------
            # Each yield point ~= one 512-cycle PE slot (a pair of concurrent
            # half-matmuls).  Pairs use complementary halves (t ^ i) so the
            # two in-flight matmuls sit in different row groups AND write
            # different PSUM banks.

            def qk_job(sc, wsb, dest):
                """qT/kT for s-chunk sc, one head-pair at a time (holds a
                single aux-PSUM slot so normalize/other fillers can run)."""
                if not SPLIT_HALVES:
                    for hp in range(NHP):
                        acc = aux_ps.tile([P, QB], F32, tag="aux", name="qkacc")
                        for ec in range(NEC):
                            nc.tensor.matmul(
                                acc[:],
                                wsb[:, ec * HD + P * hp:ec * HD + P * (hp + 1)],
                                xT[ec][:, QB * sc:QB * (sc + 1)],
                                start=(ec == 0), stop=(ec == NEC - 1),
                            )
                            if ec % 2 == 1:
                                yield
                        nc.vector.tensor_copy(
                            dest[hp][:, QB * sc:QB * (sc + 1)], acc[:])
                    return
                acc = [aux_ps.tile([P, QB], F32, tag="aux", name="qkacc")
                       for _ in range(NHP)]
                if SPLIT_HALVES:
                    for ec in range(NEC):
                        for t in range(2):
                            for hp in range(NHP):
                                half = t ^ hp
                                nc.tensor.matmul(
                                    acc[hp][:],
                                    wsb[H * half:H * (half + 1),
                                        ec * HD + P * hp:ec * HD + P * (hp + 1)],
                                    xT[ec][H * half:H * (half + 1),
                                           QB * sc:QB * (sc + 1)],
                                    start=(ec == 0 and t == 0),
                                    stop=(ec == NEC - 1 and t == 1),
                                    tile_position=(H * half, 0),
                                )
                            yield
                else:
                    for ec in range(NEC):
                        for hp in range(NHP):
                            nc.tensor.matmul(
                                acc[hp][:],
                                wsb[:, ec * HD + P * hp:ec * HD + P * (hp + 1)],
                                xT[ec][:, QB * sc:QB * (sc + 1)],
                                start=(ec == 0), stop=(ec == NEC - 1),
                            )
                        yield
                for hp in range(NHP):
                    nc.vector.tensor_copy(
                        dest[hp][:, QB * sc:QB * (sc + 1)], acc[hp][:])

            def v_job(st0):
                """v (natural layout) for s-tiles st0, st0+1."""
                if not SPLIT_HALVES:
                    for i in range(2):
                        st = st0 + i
                        vp = aux_ps.tile([P, HD], F32, tag="aux", name="vp")
                        for ec in range(NEC):
                            nc.tensor.matmul(
                                vp[:],
                                xT[ec][:, P * st:P * (st + 1)],
                                wv_sb[:, ec * HD:(ec + 1) * HD],
                                start=(ec == 0), stop=(ec == NEC - 1),
                            )
                            if ec % 4 == 3:
                                yield
                        vslab = v_ext[:, NH * VW * st:NH * VW * (st + 1)]
                        nc.vector.tensor_copy(
                            vslab.rearrange("p (h c) -> p h c", h=NH)[:, :, 0:D],
                            vp.rearrange("p (h c) -> p h c", h=NH),
                        )
                    return
                vp = [aux_ps.tile([P, HD], F32, tag="aux", name="vp")
                      for _ in range(2)]
                if SPLIT_HALVES:
                    for ec in range(NEC):
                        for t in range(2):
                            for i in range(2):
                                half = t ^ i
                                st = st0 + i
                                nc.tensor.matmul(
                                    vp[i][:],
                                    xT[ec][H * half:H * (half + 1),
                                           P * st:P * (st + 1)],
                                    wv_sb[H * half:H * (half + 1),
                                          ec * HD:(ec + 1) * HD],
                                    start=(ec == 0 and t == 0),
                                    stop=(ec == NEC - 1 and t == 1),
                                    tile_position=(H * half, 0),
                                )
                            yield
                else:
                    for ec in range(NEC):
                        for i in range(2):
                            st = st0 + i
                            nc.tensor.matmul(
                                vp[i][:],
                                xT[ec][:, P * st:P * (st + 1)],
                                wv_sb[:, ec * HD:(ec + 1) * HD],
                                start=(ec == 0), stop=(ec == NEC - 1),
                            )
                        yield
                for i in range(2):
                    st = st0 + i
                    vslab = v_ext[:, NH * VW * st:NH * VW * (st + 1)]
                    nc.vector.tensor_copy(
                        vslab.rearrange("p (h c) -> p h c", h=NH)[:, :, 0:D],
                        vp[i].rearrange("p (h c) -> p h c", h=NH),
                    )

            def t_job(sc):
                """PE transposes: xn_all chunk sc -> xT[:][:, chunk sc]."""
                xv = xn_all.rearrange("p (k e) -> p k e", e=E)
                for ec in range(NEC):
                    pt = aux_ps.tile([P, QB], F16, tag="aux", name="pt")
                    for k in range(4):
                        nc.tensor.transpose(
                            pt[:, P * k:P * (k + 1)],
                            xv[:, 4 * sc + k, P * ec:P * (ec + 1)], ident)
                    nc.vector.tensor_copy(
                        xT[ec][:, QB * sc:QB * (sc + 1)], pt[:])
                    yield

            def proj_job(qt, ceng=None):
                """y[qt-tile] = sum_hp oT_all[hp][:, qt].T @ wp[hp]."""
                ceng = ceng or nc.vector
                if not SPLIT_HALVES:
                    ysb = y_pool.tile([P, E], F16, tag="ysb", name="ysb")
                    for nk in range(2):
                        pj = aux_ps.tile([P, 512], F32, tag="aux", name="pj")
                        for hp in range(NHP):
                            nc.tensor.matmul(
                                pj[:],
                                oT_all[hp][:, P * qt:P * (qt + 1)],
                                wp_sb[hp][:, 512 * nk:512 * (nk + 1)],
                                start=(hp == 0), stop=(hp == NHP - 1),
                            )
                        yield
                        if ceng is nc.scalar:
                            ceng.copy(ysb[:, 512 * nk:512 * (nk + 1)], pj[:])
                        else:
                            ceng.tensor_copy(
                                ysb[:, 512 * nk:512 * (nk + 1)], pj[:])
                    nc.sync.dma_start(out=y_d[P * qt:P * (qt + 1), :], in_=ysb)
                    return
                pj = [aux_ps.tile([P, 512], F32, tag="aux", name="pj")
                      for _ in range(2)]
                if SPLIT_HALVES:
                    for hp in range(NHP):
                        for t in range(2):
                            for nk in range(2):
                                half = t ^ nk
                                nc.tensor.matmul(
                                    pj[nk][:],
                                    oT_all[hp][H * half:H * (half + 1),
                                               P * qt:P * (qt + 1)],
                                    wp_sb[hp][H * half:H * (half + 1),
                                              512 * nk:512 * (nk + 1)],
                                    start=(hp == 0 and t == 0),
                                    stop=(hp == NHP - 1 and t == 1),
                                    tile_position=(H * half, 0),
                                )
                            yield
                else:
                    for nk in range(2):
                        for hp in range(NHP):
                            nc.tensor.matmul(
                                pj[nk][:],
                                oT_all[hp][:, P * qt:P * (qt + 1)],
                                wp_sb[hp][:, 512 * nk:512 * (nk + 1)],
                                start=(hp == 0), stop=(hp == NHP - 1),
                            )
                        yield
                ysb = y_pool.tile([P, E], F16, tag="ysb", name="ysb")
                for nk in range(2):
                    if ceng is nc.scalar:
                        ceng.copy(ysb[:, 512 * nk:512 * (nk + 1)], pj[nk][:])
                    else:
                        ceng.tensor_copy(
                            ysb[:, 512 * nk:512 * (nk + 1)], pj[nk][:])
                nc.sync.dma_start(out=y_d[P * qt:P * (qt + 1), :], in_=ysb)

            # FIFO of (chunk_tag, generator); drained as PE filler.
            jobs = []

            def chunk_jobs(c):
                # qk due at unit (c,0) start (tag 10c); v slabs of chunk c are
                # only read by the last 4 j-tiles of units (c,*), so they can
                # drain as fillers until then (tag 10c+5).
                pre = [] if XT_DMA_TRANSPOSE else [(10 * c, t_job(c))]
                return pre + [
                    (10 * c, qk_job(c, wq_sb, qT)),
                    (10 * c, qk_job(c, wk_sb, kT)),
                    (10 * c + 5, v_job(4 * c)), (10 * c + 5, v_job(4 * c + 2))]

            def drain(n):
                k = 0
                while jobs and k < n:
                    try:
                        next(jobs[0][1])
                        k += 1
                    except StopIteration:
                        jobs.pop(0)

            def flush_tag(c):
                while jobs and jobs[0][0] <= c:
                    try:
                        next(jobs[0][1])
                    except StopIteration:
                        jobs.pop(0)

            def exhaust(gen):
                for _ in gen:
                    pass

            # ---------------- chunk 0 (no attention to overlap) ----------------
            for _, g in chunk_jobs(0):
                exhaust(g)

            # ---------------- attention units, pipelined with chunks 1..3 ----
            for qb in range(NQB):
                for hp in range(NHP):
                    if hp == 0:
                        if qb + 1 < NQB:
                            jobs.extend(chunk_jobs(qb + 1))
                        flush_tag(10 * qb)  # chunk qb qT/kT must be emitted
                    n_j = (qb + 1) * JPQ
                    oT2 = [oT_ps.tile([P, 512], F32, tag="oT", name="oT")
                           for _ in range(2)]
                    pend = None
                    for js in range(n_j):
                        if js == JPQ * qb and qb > 0:
                            flush_tag(10 * qb + 5)  # v slabs of chunk qb
                        cm = max(0, P * js - QB * qb)  # first visible q col
                        is_diag = js >= JPQ * qb
                        sT = sT_ps.tile([P, 1024], F32, tag="sT", name="sT")
                        for h in range(2):
                            nc.tensor.matmul(
                                sT[:, 512 * h + cm:512 * (h + 1)],
                                kT[hp][H * h:H * (h + 1), P * js:P * (js + 1)],
                                qT[hp][H * h:H * (h + 1),
                                       QB * qb + cm:QB * (qb + 1)],
                                start=True, stop=True,
                                tile_position=(H * h, 0),
                            )
                        pT = pT_pool.tile([P, 1024], F16, tag="pT", name="pT")
                        if cm == 0:
                            nc.scalar.activation(pT[:], sT[:], EXP, scale=0.125)
                        else:
                            nc.scalar.activation(
                                pT.rearrange("p (h c) -> p h c", h=2)[:, :, cm:512],
                                sT.rearrange("p (h c) -> p h c", h=2)[:, :, cm:512],
                                EXP, scale=0.125)
                        if is_diag:
                            # fp16 SBUF-only multiply: runs on the (idle)
                            # gpsimd engine, off the DVE critical path
                            for h in range(2):
                                lo = 512 * h + cm
                                nc.gpsimd.tensor_mul(
                                    pT[:, lo:lo + P], pT[:, lo:lo + P], tri[:])
                        if pend is not None:
                            pend()
                        drain(3 if qb else 2)

                        def make_pv(js=js, cm=cm, pT=pT, n_j=n_j):
                            def emit():
                                if SPLIT_HALVES:
                                    for t in range(2):
                                        for h in range(2):
                                            half = t ^ h
                                            hl = 2 * hp + h
                                            nc.tensor.matmul(
                                                oT2[h][0:VW, cm:512],
                                                v_ext[H * half:H * (half + 1),
                                                      NH * VW * js + VW * hl:
                                                      NH * VW * js + VW * (hl + 1)],
                                                pT[H * half:H * (half + 1),
                                                   512 * h + cm:512 * (h + 1)],
                                                start=(js == 0 and t == 0),
                                                stop=(js == n_j - 1 and t == 1),
                                                tile_position=(H * half, 0),
                                            )
                                else:
                                    for h in range(2):
                                        hl = 2 * hp + h
                                        nc.tensor.matmul(
                                            oT2[h][0:VW, cm:512],
                                            v_ext[:, NH * VW * js + VW * hl:
                                                  NH * VW * js + VW * (hl + 1)],
                                            pT[:, 512 * h + cm:512 * (h + 1)],
                                            start=(js == 0),
                                            stop=(js == n_j - 1),
                                        )
                            return emit
                        pend = make_pv()
                    pend()
                    # normalize: rows 0:64 of oT2 = numerator, row 64 = sum.
                    # One wide copy evacuates the whole oT2 bank (frees the
                    # PSUM slot for the next unit ASAP); the denominator row
                    # is then broadcast with a 1-row outer-product on the PE
                    # and reciprocal'd 64-lanes-wide on DVE.
                    osb = [None, None]
                    rb = [None, None]
                    for h in range(2):
                        osb[h] = o_pool.tile([P, 512], F16, tag="osb",
                                             name="osb")
                        nc.vector.tensor_copy(osb[h][0:VW, :], oT2[h][0:VW, :])
                        rb_ps = aux_ps.tile([P, 512], F32, tag="aux",
                                            name="rbps")
                        nc.tensor.matmul(
                            rb_ps[0:D, :], ones16[D:D + 1, :],
                            osb[h][D:D + 1, :], start=True, stop=True,
                            tile_position=(D, 0))
                        rb[h] = rb_pool.tile([D, 512], F32, tag="rb", name="rb")
                        nc.vector.reciprocal_approx_fast(rb[h][:], rb_ps[0:D, :])
                    last = (qb == NQB - 1 and hp == NHP - 1)
                    if not last:
                        for h in range(2):
                            nc.vector.tensor_mul(
                                oT_all[hp][D * h:D * (h + 1),
                                           QB * qb:QB * (qb + 1)],
                                osb[h][0:D, :], rb[h][:])
                    else:
                        # final unit: normalize per q-tile and chase each with
                        # its projection so the tail pipelines
                        for qi in range(JPQ):
                            for h in range(2):
                                nc.vector.tensor_mul(
                                    oT_all[hp][D * h:D * (h + 1),
                                               QB * qb + P * qi:
                                               QB * qb + P * (qi + 1)],
                                    osb[h][0:D, P * qi:P * (qi + 1)],
                                    rb[h][:, P * qi:P * (qi + 1)])
                            exhaust(proj_job(JPQ * qb + qi, ceng=nc.scalar))
                if qb < NQB - 1:
                    jobs.extend((10 * qb + 6, proj_job(qt))
                                for qt in range(JPQ * qb, JPQ * (qb + 1)))
            flush_tag(1000)  # drain remaining projections

    if lower_isa:
        nc.finalize()
    return nc


_CACHED_NC = None


def _get_nc():
    global _CACHED_NC
    if _CACHED_NC is None:
        _CACHED_NC = build_core_program()
    return _CACHED_NC


def shard_inputs(x, Wq, Wk, Wv, Wp):
    in_maps = []
    x16 = [np.ascontiguousarray(x[b], dtype=np.float16) for b in range(B)]
    cident = np.eye(P, dtype=np.float16)
    ctri = np.triu(np.ones((P, P), dtype=np.float16))  # keep col >= row
    cones = np.ones((P, NST * NH), dtype=np.float16)
    for core in range(N_CORES):
        b, g = core // GROUPS, core % GROUPS
        sl = slice(HD * g, HD * (g + 1))
        in_maps.append({
            "x": x16[b],
            "wq": np.ascontiguousarray(Wq[:, sl]).astype(np.float16),
            "wk": np.ascontiguousarray(Wk[:, sl]).astype(np.float16),
            "wv": np.ascontiguousarray(Wv[:, sl]).astype(np.float16),
            "wp": np.ascontiguousarray(Wp[sl, :]).astype(np.float16),
            "cident": cident, "ctri": ctri, "cones": cones,
        })
    return in_maps


def _ensure_ntff_hook():
    """Provide antenv.axon_hooks (missing in this image) so trace=True can
    collect NTFF profiles through libaxon_pjrt's nrt-profile C ABI."""
    import types
    try:
        from antenv.axon_hooks import get_axon_ntff_profile_hook  # noqa: F401
        return
    except ImportError:
        pass
    import antenv
    mod = types.ModuleType("antenv.axon_hooks")
    mod._hook = None
    def set_axon_ntff_profile_hook(h):
        mod._hook = h
    def get_axon_ntff_profile_hook():
        return mod._hook
    mod.set_axon_ntff_profile_hook = set_axon_ntff_profile_hook
    mod.get_axon_ntff_profile_hook = get_axon_ntff_profile_hook
    sys.modules["antenv.axon_hooks"] = mod
    antenv.axon_hooks = mod
    try:
        from trn_agent_boot.trn_boot import _ntff_profile_via_ctypes
        mod._hook = _ntff_profile_via_ctypes("/opt/axon/libaxon_pjrt.so")
    except Exception as e:  # degrade: tracing skipped, run still works
        print(f"ntff hook setup failed: {e}", file=sys.stderr)


def run(inputs, trace=False, **spmd_kwargs):
    """Returns (full_output [B,S,E], BassKernelResults)."""
    from concourse.bass_utils import run_bass_kernel_spmd
    if trace:
        _ensure_ntff_hook()
    x = np.asarray(inputs["x"], dtype=np.float32)
    Wq = np.asarray(inputs["Wq"], dtype=np.float32)
    Wk = np.asarray(inputs["Wk"], dtype=np.float32)
    Wv = np.asarray(inputs["Wv"], dtype=np.float32)
    Wp = np.asarray(inputs["Wp"], dtype=np.float32)
    bp = np.asarray(inputs["bp"], dtype=np.float32)

    nc = _get_nc()
    in_maps = shard_inputs(x, Wq, Wk, Wv, Wp)
    res = run_bass_kernel_spmd(nc, in_maps, list(range(N_CORES)),
                               trace=trace, **spmd_kwargs)
    out = np.zeros((B, S, E), dtype=np.float32)
    for core in range(N_CORES):
        out[core // GROUPS] += res.results[core]["y"].astype(np.float32)
    out += bp[None, None, :]
    return out, res


def kernel(x, Wq, Wk, Wv, Wp, bp):
    out, _ = run({"x": x, "Wq": Wq, "Wk": Wk, "Wv": Wv, "Wp": Wp, "bp": bp})
    return out


# revision 45
# speedup vs baseline: 1.0407x; 1.0407x over previous
"""Causal multi-head attention (B=2, S=2048, E=1024, H=16, D=64) on 8 trn2 NeuronCores.

Sharding: core c handles batch b = c // 4 and head group g = c % 4 (4 heads each).
Each core computes, for its batch and its 4 heads:
    q/k/v = x @ W[qkv][:, 256g:256g+256], causal attention, then the partial
    projection  out_heads @ Wp[256g:256g+256, :]  -> [2048, 1024].
Host gathers: out[b] = sum_g partial[b, g] + bp  (the "all-reduce" of the TP hint).

v2 design notes (vs the f32r baseline):
- All matmul operands are fp16 (x and W are converted to fp16 on the host, the
  partial y is returned fp16 and reduced in fp32 on the host). 10-bit mantissa
  matches f32r precision; halves all DMA traffic.
- xT ([E,S] layout) is produced by XBAR DMA-transpose straight from DRAM, so
  the PE never runs transposes and there is no x-natural staging.
- Every matmul in the program runs in the 64x128 row-tiled PE mode: each
  logical 128-deep contraction is split into two 64-row halves placed at
  tile_position (0,0)/(64,0).  Pairs of matmuls that target different PSUM
  banks are emitted with complementary halves so the PE executes them
  concurrently (measured on HW: same-timestamp row_grp h0/h64 pairs), which
  keeps per-slot cost equal to the untiled version while avoiding any
  tiling-mode switch (mode switches drain the array).
- The Act engine only runs exp (the serial ~60us resource); scores for both
  heads of a pair land in one [128,1024] PSUM tile so each j-tile needs one
  ACTIVATE.  Causal masking of diagonal tiles is a DVE multiply with a
  constant lower-triangle fp16 mask; QK/exp/PV all stream only [cm, 512) of
  each diagonal block.
- QKV for s-chunk c and attention for q-block c-1 are software-pipelined:
  chunk jobs (and later the output projection) are generators that yield
  ~512-cycle slices, drained as PE filler between attention tiles so the PE
  never idles while Act churns through exp.
"""

import os
import sys
import numpy as np

sys.path.insert(0, "/opt/trn_rl_repo")

import concourse.bass as bass
import concourse.bacc as bacc_mod
import concourse.mybir as mybir
import concourse.tile as tile
from concourse import library_config

F32 = mybir.dt.float32
F16 = mybir.dt.float16
P = 128
H = 64                # contraction half (row-tile height)

B = 2
S = 2048
E = 1024
NHEADS_TOTAL = 16
D = 64
N_CORES = 8
GROUPS = 4            # head groups (tensor parallel)
HD = NHEADS_TOTAL * D // GROUPS   # 256 head-dims per core
NH = HD // D          # heads per core (4)
NHP = HD // P         # head pairs (2)
NST = S // P          # s tiles (16)
NEC = E // P          # e chunks (8)
QB = 512              # q-block width
NQB = S // QB         # q blocks (4)
JPQ = QB // P         # j tiles per q block (4)
VW = D + 1            # 65: v columns + ones column

EXP = mybir.ActivationFunctionType.Exp

# debug toggles
# DMA-transpose is functionally fine but each issue instruction occupies the
# SP engine ~1.3us (descriptor gen) — 32 issues poison the whole schedule.
# PE transposes in fp16 cost 53ns/tile and ride in idle PE slots instead.
XT_DMA_TRANSPOSE = os.environ.get("KQ_XT_DMA", "0") == "1"
# half-split row tiling hangs on HW (same-bank access from two row groups);
# full-contraction matmuls cost the same cycles anyway.
SPLIT_HALVES = os.environ.get("KQ_SPLIT", "0") == "1"


def build_core_program(lower_isa=True):
    """One NeuronCore's program (SPMD: all 8 cores run this on different data)."""
    nc = bacc_mod.Bacc()
    x_d = nc.declare_dram_parameter("x", [S, E], F16, False)
    wq_d = nc.declare_dram_parameter("wq", [E, HD], F16, False)
    wk_d = nc.declare_dram_parameter("wk", [E, HD], F16, False)
    wv_d = nc.declare_dram_parameter("wv", [E, HD], F16, False)
    wp_d = nc.declare_dram_parameter("wp", [HD, E], F16, False)
    # host-built constants: identity (PE transposes), causal keep-mask
    # (tri[r,c] = c>=r), all-ones (v_ext denominator column + the
    # denominator-broadcast outer product). Building these on-device costs
    # ~4us of gpsimd iota on the critical path; a DMA is ~100ns.
    id_d = nc.declare_dram_parameter("cident", [P, P], F16, False)
    tri_d = nc.declare_dram_parameter("ctri", [P, P], F16, False)
    one_d = nc.declare_dram_parameter("cones", [P, NST * NH], F16, False)
    y_d = nc.declare_dram_parameter("y", [S, E], F16, True)

    with tile.TileContext(nc) as tc:
        # "standard" provides InstIota (make_identity) + InstTensorTensor
        # (diag-mask multiplies on the pool engine)
        nc.gpsimd.load_library(library_config.standard)
        from contextlib import ExitStack
        with ExitStack() as ctx:
            const = ctx.enter_context(tc.tile_pool(name="const", bufs=1))
            persist = ctx.enter_context(tc.tile_pool(name="persist", bufs=1))

            # ---------------- persistent SBUF tiles ----------------
            xT = [persist.tile([P, S], F16, tag=f"xT{ec}", name=f"xT{ec}")
                  for ec in range(NEC)]
            qT = [persist.tile([P, S], F16, tag=f"qT{hp}", name=f"qT{hp}")
                  for hp in range(NHP)]
            kT = [persist.tile([P, S], F16, tag=f"kT{hp}", name=f"kT{hp}")
                  for hp in range(NHP)]
            v_ext = persist.tile([P, NST * NH * VW], F16, tag="v_ext",
                                 name="v_ext")
            oT_all = [persist.tile([P, S], F16, tag=f"oT{hp}", name=f"oT{hp}")
                      for hp in range(NHP)]
            wq_sb = persist.tile([P, NEC * HD], F16, tag="wq", name="wq_sb")
            wk_sb = persist.tile([P, NEC * HD], F16, tag="wk", name="wk_sb")
            wv_sb = persist.tile([P, NEC * HD], F16, tag="wv", name="wv_sb")
            wp_sb = [persist.tile([P, E], F16, tag=f"wp{hp}", name=f"wp{hp}")
                     for hp in range(NHP)]
            xn_all = None
            if not XT_DMA_TRANSPOSE:
                # x natural staging for the PE transposes
                xn_all = persist.tile([P, NST * E], F16, tag="xn", name="xn_all")
            ident = const.tile([P, P], F16)
            tri = const.tile([P, P], F16)
            ones16 = const.tile([P, D], F16)

            # ---------------- DMA (all prefetched up front) ----------------
            # chunk-0 x first (it gates the whole pipeline), split across the
            # two HWDGE issuing engines (SP + Activation); then wq/wk (needed
            # with chunk 0), then the rest.
            def xdma_t(sc, ec):
                nc.sync.dma_start_transpose(
                    xT[ec][:, QB * sc:QB * (sc + 1)],
                    x_d[QB * sc:QB * (sc + 1), P * ec:P * (ec + 1)],
                )

            def xdma_n(k):  # one [128, E] row-tile of x into staging
                nc.sync.dma_start(
                    out=xn_all.rearrange("p (k e) -> p k e", e=E)[:, k:k + 1, :],
                    in_=x_d[P * k:P * (k + 1), :].rearrange(
                        "(k p) e -> p k e", p=P),
                )

            def wdma(wd, wsb):
                nc.sync.dma_start(
                    out=wsb.rearrange("p (c n) -> p c n", c=NEC),
                    in_=wd[:, :].rearrange("(c p) n -> p c n", p=P),
                )

            # ident gates the transposes, wq the first qk matmuls — first.
            # Chunk 0 of x arrives natural (PE-transposed: fast to start);
            # chunks 1-3 arrive via XBAR DMA-transpose straight into xT —
            # each issue costs ~1.3us of SP time but removes 4 PE transposes
            # + a DVE copy, and the SP engine is otherwise idle.
            nc.sync.dma_start(out=ident, in_=id_d[:, :])
            wdma(wq_d, wq_sb)
            for k in range(4):
                xdma_n(k)
            nc.sync.dma_start(out=tri, in_=tri_d[:, :])
            nc.sync.dma_start(out=ones16, in_=one_d[:, 0:D])
            # denominator ones-column of every v slab, straight from DRAM
            nc.sync.dma_start(
                out=v_ext.rearrange("p (s c) -> p s c", c=VW)[:, :, D:VW],
                in_=one_d[:, :].rearrange("p (s o) -> p s o", o=1),
            )
            wdma(wk_d, wk_sb)
            wdma(wv_d, wv_sb)
            for hp in range(NHP):
                nc.sync.dma_start(out=wp_sb[hp], in_=wp_d[P * hp:P * (hp + 1), :])
            for sc in range(1, NQB):
                for ec in range(NEC):
                    xdma_t(sc, ec)

            # warm up the Act exp table during the DMA wait
            warm = const.tile([1, 1], F32)
            nc.scalar.activation(warm[0:1, 0:1], ones16[0:1, 0:1], EXP,
                                 scale=1.0)

            # ---------------- PSUM pools ----------------
            # 8 banks total: sT 2x[128,1024]f32 (4) + oT 2x[128,512]f32 (2)
            # + aux 2x[128,512]f32 (2).
            sT_ps = ctx.enter_context(
                tc.tile_pool(name="sT_ps", bufs=2, space="PSUM"))
            oT_ps = ctx.enter_context(
                tc.tile_pool(name="oT_ps", bufs=2, space="PSUM"))
            aux_ps = ctx.enter_context(
                tc.tile_pool(name="aux_ps", bufs=2, space="PSUM"))
            pT_pool = ctx.enter_context(tc.tile_pool(name="pT", bufs=4))
            y_pool = ctx.enter_context(tc.tile_pool(name="ysb", bufs=2))
            o_pool = ctx.enter_context(tc.tile_pool(name="osb", bufs=4))
            rb_pool = ctx.enter_context(tc.tile_pool(name="rbp", bufs=4))

            # ---------------- filler job generators ----------------
            # Each yield point ~= one 512-cycle PE slot (a pair of concurrent
            # half-matmuls).  Pairs use complementary halves (t ^ i) so the
            # two in-flight matmuls sit in different row groups AND write
            # different PSUM banks.

            def qk_job(sc, wsb, dest):
                """qT/kT for s-chunk sc, one head-pair at a time (holds a
                single aux-PSUM slot so normalize/other fillers can run)."""
                if not SPLIT_HALVES:
                    for hp in range(NHP):
                        acc = aux_ps.tile([P, QB], F32, tag="aux", name="qkacc")
                        for ec in range(NEC):
                            nc.tensor.matmul(
                                acc[:],
                                wsb[:, ec * HD + P * hp:ec * HD + P * (hp + 1)],
                                xT[ec][:, QB * sc:QB * (sc + 1)],
                                start=(ec == 0), stop=(ec == NEC - 1),
                            )
                            if ec % 2 == 1:
                                yield
                        nc.vector.tensor_copy(
                            dest[hp][:, QB * sc:QB * (sc + 1)], acc[:])
                    return
                acc = [aux_ps.tile([P, QB], F32, tag="aux", name="qkacc")
                       for _ in range(NHP)]
                if SPLIT_HALVES:
                    for ec in range(NEC):
                        for t in range(2):
                            for hp in range(NHP):
                                half = t ^ hp
                                nc.tensor.matmul(
                                    acc[hp][:],
                                    wsb[H * half:H * (half + 1),
                                        ec * HD + P * hp:ec * HD + P * (hp + 1)],
                                    xT[ec][H * half:H * (half + 1),
                                           QB * sc:QB * (sc + 1)],
                                    start=(ec == 0 and t == 0),
                                    stop=(ec == NEC - 1 and t == 1),
                                    tile_position=(H * half, 0),
                                )
                            yield
                else:
                    for ec in range(NEC):
                        for hp in range(NHP):
                            nc.tensor.matmul(
                                acc[hp][:],
                                wsb[:, ec * HD + P * hp:ec * HD + P * (hp + 1)],
                                xT[ec][:, QB * sc:QB * (sc + 1)],
                                start=(ec == 0), stop=(ec == NEC - 1),
                            )
                        yield
                for hp in range(NHP):
                    nc.vector.tensor_copy(
                        dest[hp][:, QB * sc:QB * (sc + 1)], acc[hp][:])

            def v_job(st0):
                """v (natural layout) for s-tiles st0, st0+1."""
                if not SPLIT_HALVES:
                    for i in range(2):
                        st = st0 + i
                        vp = aux_ps.tile([P, HD], F32, tag="aux", name="vp")
                        for ec in range(NEC):
                            nc.tensor.matmul(
                                vp[:],
                                xT[ec][:, P * st:P * (st + 1)],
                                wv_sb[:, ec * HD:(ec + 1) * HD],
                                start=(ec == 0), stop=(ec == NEC - 1),
                            )
                            if ec % 4 == 3:
                                yield
                        vslab = v_ext[:, NH * VW * st:NH * VW * (st + 1)]
                        nc.vector.tensor_copy(
                            vslab.rearrange("p (h c) -> p h c", h=NH)[:, :, 0:D],
                            vp.rearrange("p (h c) -> p h c", h=NH),
                        )
                    return
                vp = [aux_ps.tile([P, HD], F32, tag="aux", name="vp")
                      for _ in range(2)]
                if SPLIT_HALVES:
                    for ec in range(NEC):
                        for t in range(2):
                            for i in range(2):
                                half = t ^ i
                                st = st0 + i
                                nc.tensor.matmul(
                                    vp[i][:],
                                    xT[ec][H * half:H * (half + 1),
                                           P * st:P * (st + 1)],
                                    wv_sb[H * half:H * (half + 1),
                                          ec * HD:(ec + 1) * HD],
                                    start=(ec == 0 and t == 0),
                                    stop=(ec == NEC - 1 and t == 1),
                                    tile_position=(H * half, 0),
                                )
                            yield
                else:
                    for ec in range(NEC):
                        for i in range(2):
                            st = st0 + i
                            nc.tensor.matmul(
                                vp[i][:],
                                xT[ec][:, P * st:P * (st + 1)],
                                wv_sb[:, ec * HD:(ec + 1) * HD],
                                start=(ec == 0), stop=(ec == NEC - 1),
                            )
                        yield
                for i in range(2):
                    st = st0 + i
                    vslab = v_ext[:, NH * VW * st:NH * VW * (st + 1)]
                    nc.vector.tensor_copy(
                        vslab.rearrange("p (h c) -> p h c", h=NH)[:, :, 0:D],
                        vp[i].rearrange("p (h c) -> p h c", h=NH),
                    )

            def t_job(sc):
                """PE transposes: xn_all chunk sc -> xT[:][:, chunk sc]."""
                xv = xn_all.rearrange("p (k e) -> p k e", e=E)
                for ec in range(NEC):
                    pt = aux_ps.tile([P, QB], F16, tag="aux", name="pt")
                    for k in range(4):
                        nc.tensor.transpose(
                            pt[:, P * k:P * (k + 1)],
                            xv[:, 4 * sc + k, P * ec:P * (ec + 1)], ident)
                    nc.vector.tensor_copy(
                        xT[ec][:, QB * sc:QB * (sc + 1)], pt[:])
                    yield

            def proj_job(qt, ceng=None):
                """y[qt-tile] = sum_hp oT_all[hp][:, qt].T @ wp[hp]."""
                ceng = ceng or nc.vector
                if not SPLIT_HALVES:
                    ysb = y_pool.tile([P, E], F16, tag="ysb", name="ysb")
                    for nk in range(2):
                        pj = aux_ps.tile([P, 512], F32, tag="aux", name="pj")
                        for hp in range(NHP):
                            nc.tensor.matmul(
                                pj[:],
                                oT_all[hp][:, P * qt:P * (qt + 1)],
                                wp_sb[hp][:, 512 * nk:512 * (nk + 1)],
                                start=(hp == 0), stop=(hp == NHP - 1),
                            )
                        yield
                        if ceng is nc.scalar:
                            ceng.copy(ysb[:, 512 * nk:512 * (nk + 1)], pj[:])
                        else:
                            ceng.tensor_copy(
                                ysb[:, 512 * nk:512 * (nk + 1)], pj[:])
                    nc.sync.dma_start(out=y_d[P * qt:P * (qt + 1), :], in_=ysb)
                    return
                pj = [aux_ps.tile([P, 512], F32, tag="aux", name="pj")
                      for _ in range(2)]
                if SPLIT_HALVES:
                    for hp in range(NHP):
                        for t in range(2):
                            for nk in range(2):
                                half = t ^ nk
                                nc.tensor.matmul(
                                    pj[nk][:],
                                    oT_all[hp][H * half:H * (half + 1),
                                               P * qt:P * (qt + 1)],
                                    wp_sb[hp][H * half:H * (half + 1),
                                              512 * nk:512 * (nk + 1)],
                                    start=(hp == 0 and t == 0),
                                    stop=(hp == NHP - 1 and t == 1),
                                    tile_position=(H * half, 0),
                                )
                            yield
                else:
                    for nk in range(2):
                        for hp in range(NHP):
                            nc.tensor.matmul(
                                pj[nk][:],
                                oT_all[hp][:, P * qt:P * (qt + 1)],
                                wp_sb[hp][:, 512 * nk:512 * (nk + 1)],
                                start=(hp == 0), stop=(hp == NHP - 1),
                            )
                        yield
                ysb = y_pool.tile([P, E], F16, tag="ysb", name="ysb")
                for nk in range(2):
                    if ceng is nc.scalar:
                        ceng.copy(ysb[:, 512 * nk:512 * (nk + 1)], pj[nk][:])
                    else:
                        ceng.tensor_copy(
                            ysb[:, 512 * nk:512 * (nk + 1)], pj[nk][:])
                nc.sync.dma_start(out=y_d[P * qt:P * (qt + 1), :], in_=ysb)

            # FIFO of (chunk_tag, generator); drained as PE filler.
            jobs = []

            def chunk_jobs(c):
                # qk due at unit (c,0) start (tag 10c); v slabs of chunk c are
                # only read by the last 4 j-tiles of units (c,*), so they can
                # drain as fillers until then (tag 10c+5). Only chunk 0 is
                # PE-transposed; chunks 1-3 land in xT via DMA-transpose.
                pre = [(10 * c, t_job(c))] if c == 0 else []
                return pre + [
                    (10 * c, qk_job(c, wq_sb, qT)),
                    (10 * c, qk_job(c, wk_sb, kT)),
                    (10 * c + 5, v_job(4 * c)), (10 * c + 5, v_job(4 * c + 2))]

            def drain(n):
                k = 0
                while jobs and k < n:
                    try:
                        next(jobs[0][1])
                        k += 1
                    except StopIteration:
                        jobs.pop(0)

            def flush_tag(c):
                while jobs and jobs[0][0] <= c:
                    try:
                        next(jobs[0][1])
                    except StopIteration:
                        jobs.pop(0)

            def exhaust(gen):
                for _ in gen:
                    pass

            # ---------------- chunk 0 (no attention to overlap) ----------------
            for tag, g in chunk_jobs(0):
                if tag == 0:
                    exhaust(g)     # transposes + qT/kT
                else:
                    jobs.append((tag, g))  # v-jobs drain as early fillers

            # per-era filler drain rate (slices per j-tile): sized so the
            # filler supply lasts through the Act-heavy late eras instead of
            # bunching at unit boundaries
            RATE = (4, 2, 2, 1)

            # ---------------- attention units, pipelined with chunks 1..3 ----
            for qb in range(NQB):
                for hp in range(NHP):
                    if hp == 0:
                        if qb + 1 < NQB:
                            jobs.extend(chunk_jobs(qb + 1))
                        flush_tag(10 * qb)  # chunk qb qT/kT must be emitted
                    n_j = (qb + 1) * JPQ
                    oT2 = [oT_ps.tile([P, 512], F32, tag="oT", name="oT")
                           for _ in range(2)]
                    pend = None
                    for js in range(n_j):
                        if js == JPQ * qb and qb > 0:
                            flush_tag(10 * qb + 5)  # v slabs of chunk qb
                        cm = max(0, P * js - QB * qb)  # first visible q col
                        is_diag = js >= JPQ * qb
                        sT = sT_ps.tile([P, 1024], F32, tag="sT", name="sT")
                        for h in range(2):
                            nc.tensor.matmul(
                                sT[:, 512 * h + cm:512 * (h + 1)],
                                kT[hp][H * h:H * (h + 1), P * js:P * (js + 1)],
                                qT[hp][H * h:H * (h + 1),
                                       QB * qb + cm:QB * (qb + 1)],
                                start=True, stop=True,
                                tile_position=(H * h, 0),
                            )
                        pT = pT_pool.tile([P, 1024], F16, tag="pT", name="pT")
                        if cm == 0:
                            nc.scalar.activation(pT[:], sT[:], EXP, scale=0.125)
                        else:
                            nc.scalar.activation(
                                pT.rearrange("p (h c) -> p h c", h=2)[:, :, cm:512],
                                sT.rearrange("p (h c) -> p h c", h=2)[:, :, cm:512],
                                EXP, scale=0.125)
                        if is_diag:
                            # fp16 SBUF-only multiply: runs on the (idle)
                            # gpsimd engine, off the DVE critical path
                            for h in range(2):
                                lo = 512 * h + cm
                                nc.gpsimd.tensor_mul(
                                    pT[:, lo:lo + P], pT[:, lo:lo + P], tri[:])
                        if pend is not None:
                            pend()
                        drain(RATE[qb])

                        def make_pv(js=js, cm=cm, pT=pT, n_j=n_j):
                            def emit():
                                if SPLIT_HALVES:
                                    for t in range(2):
                                        for h in range(2):
                                            half = t ^ h
                                            hl = 2 * hp + h
                                            nc.tensor.matmul(
                                                oT2[h][0:VW, cm:512],
                                                v_ext[H * half:H * (half + 1),
                                                      NH * VW * js + VW * hl:
                                                      NH * VW * js + VW * (hl + 1)],
                                                pT[H * half:H * (half + 1),
                                                   512 * h + cm:512 * (h + 1)],
                                                start=(js == 0 and t == 0),
                                                stop=(js == n_j - 1 and t == 1),
                                                tile_position=(H * half, 0),
                                            )
                                else:
                                    for h in range(2):
                                        hl = 2 * hp + h
                                        nc.tensor.matmul(
                                            oT2[h][0:VW, cm:512],
                                            v_ext[:, NH * VW * js + VW * hl:
                                                  NH * VW * js + VW * (hl + 1)],
                                            pT[:, 512 * h + cm:512 * (h + 1)],
                                            start=(js == 0),
                                            stop=(js == n_j - 1),
                                        )
                            return emit
                        pend = make_pv()
                    pend()
                    # normalize: rows 0:64 of oT2 = numerator, row 64 = sum.
                    # One wide copy evacuates the whole oT2 bank (frees the
                    # PSUM slot for the next unit ASAP); the denominator row
                    # is then broadcast with a 1-row outer-product on the PE
                    # and reciprocal'd 64-lanes-wide on DVE.
                    osb = [None, None]
                    rb = [None, None]
                    for h in range(2):
                        osb[h] = o_pool.tile([P, 512], F16, tag="osb",
                                             name="osb")
                        nc.vector.tensor_copy(osb[h][0:VW, :], oT2[h][0:VW, :])
                        rb_ps = aux_ps.tile([P, 512], F32, tag="aux",
                                            name="rbps")
                        nc.tensor.matmul(
                            rb_ps[0:D, :], ones16[D:D + 1, :],
                            osb[h][D:D + 1, :], start=True, stop=True,
                            tile_position=(D, 0))
                        rb[h] = rb_pool.tile([D, 512], F32, tag="rb", name="rb")
                        nc.vector.reciprocal_approx_fast(rb[h][:], rb_ps[0:D, :])
                    last = (qb == NQB - 1 and hp == NHP - 1)
                    if not last:
                        for h in range(2):
                            nc.vector.tensor_mul(
                                oT_all[hp][D * h:D * (h + 1),
                                           QB * qb:QB * (qb + 1)],
                                osb[h][0:D, :], rb[h][:])
                    else:
                        # final unit: normalize per q-tile and chase each with
                        # its projection so the tail pipelines
                        for qi in range(JPQ):
                            for h in range(2):
                                nc.vector.tensor_mul(
                                    oT_all[hp][D * h:D * (h + 1),
                                               QB * qb + P * qi:
                                               QB * qb + P * (qi + 1)],
                                    osb[h][0:D, P * qi:P * (qi + 1)],
                                    rb[h][:, P * qi:P * (qi + 1)])
                            exhaust(proj_job(JPQ * qb + qi, ceng=nc.scalar))
                if qb < NQB - 1:
                    # tag beyond all guard flushes: projections drain FIFO
                    # behind chunk work, i.e. mostly in the thin qb3 era
                    jobs.extend((1000 + qb, proj_job(qt))
                                for qt in range(JPQ * qb, JPQ * (qb + 1)))
            flush_tag(9999)  # drain remaining projections

    if lower_isa:
        nc.finalize()
    return nc


_CACHED_NC = None


def _get_nc():
    global _CACHED_NC
    if _CACHED_NC is None:
        _CACHED_NC = build_core_program()
    return _CACHED_NC


def shard_inputs(x, Wq, Wk, Wv, Wp):
    in_maps = []
    x16 = [np.ascontiguousarray(x[b], dtype=np.float16) for b in range(B)]
    cident = np.eye(P, dtype=np.float16)
    ctri = np.triu(np.ones((P, P), dtype=np.float16))  # keep col >= row
    cones = np.ones((P, NST * NH), dtype=np.float16)
    for core in range(N_CORES):
        b, g = core // GROUPS, core % GROUPS
        sl = slice(HD * g, HD * (g + 1))
        in_maps.append({
            "x": x16[b],
            "wq": np.ascontiguousarray(Wq[:, sl]).astype(np.float16),
            "wk": np.ascontiguousarray(Wk[:, sl]).astype(np.float16),
            "wv": np.ascontiguousarray(Wv[:, sl]).astype(np.float16),
            "wp": np.ascontiguousarray(Wp[sl, :]).astype(np.float16),
            "cident": cident, "ctri": ctri, "cones": cones,
        })
    return in_maps


def _ensure_ntff_hook():
    """Provide antenv.axon_hooks (missing in this image) so trace=True can
    collect NTFF profiles through libaxon_pjrt's nrt-profile C ABI."""
    import types
    try:
        from antenv.axon_hooks import get_axon_ntff_profile_hook  # noqa: F401
        return
    except ImportError:
        pass
    import antenv
    mod = types.ModuleType("antenv.axon_hooks")
    mod._hook = None
    def set_axon_ntff_profile_hook(h):
        mod._hook = h
    def get_axon_ntff_profile_hook():
        return mod._hook
    mod.set_axon_ntff_profile_hook = set_axon_ntff_profile_hook
    mod.get_axon_ntff_profile_hook = get_axon_ntff_profile_hook
    sys.modules["antenv.axon_hooks"] = mod
    antenv.axon_hooks = mod
    try:
        from trn_agent_boot.trn_boot import _ntff_profile_via_ctypes
        mod._hook = _ntff_profile_via_ctypes("/opt/axon/libaxon_pjrt.so")
    except Exception as e:  # degrade: tracing skipped, run still works
        print(f"ntff hook setup failed: {e}", file=sys.stderr)


def run(inputs, trace=False, **spmd_kwargs):
    """Returns (full_output [B,S,E], BassKernelResults)."""
    from concourse.bass_utils import run_bass_kernel_spmd
    if trace:
        _ensure_ntff_hook()
    x = np.asarray(inputs["x"], dtype=np.float32)
    Wq = np.asarray(inputs["Wq"], dtype=np.float32)
    Wk = np.asarray(inputs["Wk"], dtype=np.float32)
    Wv = np.asarray(inputs["Wv"], dtype=np.float32)
    Wp = np.asarray(inputs["Wp"], dtype=np.float32)
    bp = np.asarray(inputs["bp"], dtype=np.float32)

    nc = _get_nc()
    in_maps = shard_inputs(x, Wq, Wk, Wv, Wp)
    res = run_bass_kernel_spmd(nc, in_maps, list(range(N_CORES)),
                               trace=trace, **spmd_kwargs)
    out = np.zeros((B, S, E), dtype=np.float32)
    for core in range(N_CORES):
        out[core // GROUPS] += res.results[core]["y"].astype(np.float32)
    out += bp[None, None, :]
    return out, res


def kernel(x, Wq, Wk, Wv, Wp, bp):
    out, _ = run({"x": x, "Wq": Wq, "Wk": Wk, "Wv": Wv, "Wp": Wp, "bp": bp})
    return out


# revision 52
# speedup vs baseline: 1.1364x; 1.0919x over previous
"""Causal multi-head attention (B=2, S=2048, E=1024, H=16, D=64) on 8 trn2 NeuronCores.

Sharding: core c handles batch b = c // 4 and head group g = c % 4 (4 heads each).
Each core computes, for its batch and its 4 heads:
    q/k/v = x @ W[qkv][:, 256g:256g+256], causal attention, then the partial
    projection  out_heads @ Wp[256g:256g+256, :]  -> [2048, 1024].
Host gathers: out[b] = sum_g partial[b, g] + bp  (the "all-reduce" of the TP hint).

v2 design notes (vs the f32r baseline):
- All matmul operands are fp16 (x and W are converted to fp16 on the host, the
  partial y is returned fp16 and reduced in fp32 on the host). 10-bit mantissa
  matches f32r precision; halves all DMA traffic.
- xT ([E,S] layout) is produced by XBAR DMA-transpose straight from DRAM, so
  the PE never runs transposes and there is no x-natural staging.
- Every matmul in the program runs in the 64x128 row-tiled PE mode: each
  logical 128-deep contraction is split into two 64-row halves placed at
  tile_position (0,0)/(64,0).  Pairs of matmuls that target different PSUM
  banks are emitted with complementary halves so the PE executes them
  concurrently (measured on HW: same-timestamp row_grp h0/h64 pairs), which
  keeps per-slot cost equal to the untiled version while avoiding any
  tiling-mode switch (mode switches drain the array).
- The Act engine only runs exp (the serial ~60us resource); scores for both
  heads of a pair land in one [128,1024] PSUM tile so each j-tile needs one
  ACTIVATE.  Causal masking of diagonal tiles is a DVE multiply with a
  constant lower-triangle fp16 mask; QK/exp/PV all stream only [cm, 512) of
  each diagonal block.
- QKV for s-chunk c and attention for q-block c-1 are software-pipelined:
  chunk jobs (and later the output projection) are generators that yield
  ~512-cycle slices, drained as PE filler between attention tiles so the PE
  never idles while Act churns through exp.
"""

import os
import sys
import numpy as np

sys.path.insert(0, "/opt/trn_rl_repo")

import concourse.bass as bass
import concourse.bacc as bacc_mod
import concourse.mybir as mybir
import concourse.tile as tile
from concourse import library_config

F32 = mybir.dt.float32
F16 = mybir.dt.float16
P = 128
H = 64                # contraction half (row-tile height)

B = 2
S = 2048
E = 1024
NHEADS_TOTAL = 16
D = 64
N_CORES = 8
GROUPS = 4            # head groups (tensor parallel)
HD = NHEADS_TOTAL * D // GROUPS   # 256 head-dims per core
NH = HD // D          # heads per core (4)
NHP = HD // P         # head pairs (2)
NST = S // P          # s tiles (16)
NEC = E // P          # e chunks (8)
QB = 512              # q-block width
NQB = S // QB         # q blocks (4)
JPQ = QB // P         # j tiles per q block (4)
VW = D + 1            # 65: v columns + ones column

EXP = mybir.ActivationFunctionType.Exp

# debug toggles
# DMA-transpose is functionally fine but each issue instruction occupies the
# SP engine ~1.3us (descriptor gen) — 32 issues poison the whole schedule.
# PE transposes in fp16 cost 53ns/tile and ride in idle PE slots instead.
XT_DMA_TRANSPOSE = os.environ.get("KQ_XT_DMA", "0") == "1"
# half-split row tiling hangs on HW (same-bank access from two row groups);
# full-contraction matmuls cost the same cycles anyway.
SPLIT_HALVES = os.environ.get("KQ_SPLIT", "0") == "1"


def build_core_program(lower_isa=True):
    """One NeuronCore's program (SPMD: all 8 cores run this on different data)."""
    nc = bacc_mod.Bacc()
    x_d = nc.declare_dram_parameter("x", [S, E], F16, False)
    wq_d = nc.declare_dram_parameter("wq", [E, HD], F16, False)
    wk_d = nc.declare_dram_parameter("wk", [E, HD], F16, False)
    wv_d = nc.declare_dram_parameter("wv", [E, HD], F16, False)
    wp_d = nc.declare_dram_parameter("wp", [HD, E], F16, False)
    # host-built constants: identity (PE transposes), causal keep-mask
    # (tri[r,c] = c>=r), all-ones (v_ext denominator column + the
    # denominator-broadcast outer product). Building these on-device costs
    # ~4us of gpsimd iota on the critical path; a DMA is ~100ns.
    id_d = nc.declare_dram_parameter("cident", [P, P], F16, False)
    tri_d = nc.declare_dram_parameter("ctri", [P, P], F16, False)
    one_d = nc.declare_dram_parameter("cones", [P, NST * NH], F16, False)
    y_d = nc.declare_dram_parameter("y", [S, E], F16, True)

    with tile.TileContext(nc) as tc:
        # "standard" provides InstIota (make_identity) + InstTensorTensor
        # (diag-mask multiplies on the pool engine)
        nc.gpsimd.load_library(library_config.standard)
        from contextlib import ExitStack
        with ExitStack() as ctx:
            const = ctx.enter_context(tc.tile_pool(name="const", bufs=1))
            persist = ctx.enter_context(tc.tile_pool(name="persist", bufs=1))

            # ---------------- persistent SBUF tiles ----------------
            xT = [persist.tile([P, S], F16, tag=f"xT{ec}", name=f"xT{ec}")
                  for ec in range(NEC)]
            qT = [persist.tile([P, S], F16, tag=f"qT{hp}", name=f"qT{hp}")
                  for hp in range(NHP)]
            kT = [persist.tile([P, S], F16, tag=f"kT{hp}", name=f"kT{hp}")
                  for hp in range(NHP)]
            v_ext = persist.tile([P, NST * NH * VW], F16, tag="v_ext",
                                 name="v_ext")
            oT_all = [persist.tile([P, S], F16, tag=f"oT{hp}", name=f"oT{hp}")
                      for hp in range(NHP)]
            wq_sb = persist.tile([P, NEC * HD], F16, tag="wq", name="wq_sb")
            wk_sb = persist.tile([P, NEC * HD], F16, tag="wk", name="wk_sb")
            wv_sb = persist.tile([P, NEC * HD], F16, tag="wv", name="wv_sb")
            wp_sb = [persist.tile([P, E], F16, tag=f"wp{hp}", name=f"wp{hp}")
                     for hp in range(NHP)]
            xn_all = None
            if not XT_DMA_TRANSPOSE:
                # x natural staging for the PE transposes
                xn_all = persist.tile([P, NST * E], F16, tag="xn", name="xn_all")
            ident = const.tile([P, P], F16)
            tri = const.tile([P, P], F16)
            ones16 = const.tile([P, D], F16)

            # ---------------- DMA (all prefetched up front) ----------------
            # chunk-0 x first (it gates the whole pipeline), split across the
            # two HWDGE issuing engines (SP + Activation); then wq/wk (needed
            # with chunk 0), then the rest.
            def xdma_t(sc, ec):
                nc.sync.dma_start_transpose(
                    xT[ec][:, QB * sc:QB * (sc + 1)],
                    x_d[QB * sc:QB * (sc + 1), P * ec:P * (ec + 1)],
                )

            def xdma_n(k):  # one [128, E] row-tile of x into staging
                nc.sync.dma_start(
                    out=xn_all.rearrange("p (k e) -> p k e", e=E)[:, k:k + 1, :],
                    in_=x_d[P * k:P * (k + 1), :].rearrange(
                        "(k p) e -> p k e", p=P),
                )

            def wdma(wd, wsb):
                nc.sync.dma_start(
                    out=wsb.rearrange("p (c n) -> p c n", c=NEC),
                    in_=wd[:, :].rearrange("(c p) n -> p c n", p=P),
                )

            # ident gates the transposes, wq the first qk matmuls — first.
            # Chunk 0 of x arrives natural (PE-transposed: fast to start);
            # chunks 1-3 arrive via XBAR DMA-transpose straight into xT —
            # each issue costs ~1.3us of SP time but removes 4 PE transposes
            # + a DVE copy, and the SP engine is otherwise idle.
            nc.sync.dma_start(out=ident, in_=id_d[:, :])
            wdma(wq_d, wq_sb)
            for k in range(4):
                xdma_n(k)
            nc.sync.dma_start(out=tri, in_=tri_d[:, :])
            nc.sync.dma_start(out=ones16, in_=one_d[:, 0:D])
            # denominator ones-column of every v slab, straight from DRAM
            nc.sync.dma_start(
                out=v_ext.rearrange("p (s c) -> p s c", c=VW)[:, :, D:VW],
                in_=one_d[:, :].rearrange("p (s o) -> p s o", o=1),
            )
            wdma(wk_d, wk_sb)
            wdma(wv_d, wv_sb)
            for hp in range(NHP):
                nc.sync.dma_start(out=wp_sb[hp], in_=wp_d[P * hp:P * (hp + 1), :])
            for sc in range(1, NQB):
                for ec in range(NEC):
                    xdma_t(sc, ec)

            # warm up the Act exp table during the DMA wait
            warm = const.tile([1, 1], F32)
            nc.scalar.activation(warm[0:1, 0:1], ones16[0:1, 0:1], EXP,
                                 scale=1.0)

            # ---------------- PSUM pools ----------------
            # 8 banks total: sT 2x[128,1024]f32 (4) + oT 2x[128,512]f32 (2)
            # + aux 2x[128,512]f32 (2).
            sT_ps = ctx.enter_context(
                tc.tile_pool(name="sT_ps", bufs=2, space="PSUM"))
            oT_ps = ctx.enter_context(
                tc.tile_pool(name="oT_ps", bufs=2, space="PSUM"))
            aux_ps = ctx.enter_context(
                tc.tile_pool(name="aux_ps", bufs=2, space="PSUM"))
            pT_pool = ctx.enter_context(tc.tile_pool(name="pT", bufs=4))
            y_pool = ctx.enter_context(tc.tile_pool(name="ysb", bufs=2))
            o_pool = ctx.enter_context(tc.tile_pool(name="osb", bufs=4))
            rb_pool = ctx.enter_context(tc.tile_pool(name="rbp", bufs=4))

            # ---------------- filler job generators ----------------
            # Each yield point ~= one 512-cycle PE slot (a pair of concurrent
            # half-matmuls).  Pairs use complementary halves (t ^ i) so the
            # two in-flight matmuls sit in different row groups AND write
            # different PSUM banks.

            def qk_job(sc, wsb, dest):
                """qT/kT for s-chunk sc, one head-pair at a time (holds a
                single aux-PSUM slot so normalize/other fillers can run)."""
                if not SPLIT_HALVES:
                    acc2 = [aux_ps.tile([P, QB], F32, tag="aux", name="qkacc")
                            for _ in range(NHP)]
                    for ec in range(NEC):
                        for hp in range(NHP):
                            nc.tensor.matmul(
                                acc2[hp][:],
                                wsb[:, ec * HD + P * hp:ec * HD + P * (hp + 1)],
                                xT[ec][:, QB * sc:QB * (sc + 1)],
                                start=(ec == 0), stop=(ec == NEC - 1),
                            )
                        yield
                    for hp in range(NHP):
                        nc.vector.tensor_copy(
                            dest[hp][:, QB * sc:QB * (sc + 1)], acc2[hp][:])
                    return
                acc = [aux_ps.tile([P, QB], F32, tag="aux", name="qkacc")
                       for _ in range(NHP)]
                if SPLIT_HALVES:
                    for ec in range(NEC):
                        for t in range(2):
                            for hp in range(NHP):
                                half = t ^ hp
                                nc.tensor.matmul(
                                    acc[hp][:],
                                    wsb[H * half:H * (half + 1),
                                        ec * HD + P * hp:ec * HD + P * (hp + 1)],
                                    xT[ec][H * half:H * (half + 1),
                                           QB * sc:QB * (sc + 1)],
                                    start=(ec == 0 and t == 0),
                                    stop=(ec == NEC - 1 and t == 1),
                                    tile_position=(H * half, 0),
                                )
                            yield
                else:
                    for ec in range(NEC):
                        for hp in range(NHP):
                            nc.tensor.matmul(
                                acc[hp][:],
                                wsb[:, ec * HD + P * hp:ec * HD + P * (hp + 1)],
                                xT[ec][:, QB * sc:QB * (sc + 1)],
                                start=(ec == 0), stop=(ec == NEC - 1),
                            )
                        yield
                for hp in range(NHP):
                    nc.vector.tensor_copy(
                        dest[hp][:, QB * sc:QB * (sc + 1)], acc[hp][:])

            def v_job(st0):
                """v (natural layout) for s-tiles st0, st0+1."""
                if not SPLIT_HALVES:
                    # per-st copy right after its matmuls so PV can consume
                    # slab st while st+1 still accumulates
                    for i in range(2):
                        st = st0 + i
                        vp = aux_ps.tile([P, HD], F32, tag="aux", name="vp")
                        for ec in range(NEC):
                            nc.tensor.matmul(
                                vp[:],
                                xT[ec][:, P * st:P * (st + 1)],
                                wv_sb[:, ec * HD:(ec + 1) * HD],
                                start=(ec == 0), stop=(ec == NEC - 1),
                            )
                            if ec % 2 == 1:
                                yield
                        vslab = v_ext[:, NH * VW * st:NH * VW * (st + 1)]
                        nc.vector.tensor_copy(
                            vslab.rearrange("p (h c) -> p h c", h=NH)[:, :, 0:D],
                            vp.rearrange("p (h c) -> p h c", h=NH),
                        )
                    return
                vp = [aux_ps.tile([P, HD], F32, tag="aux", name="vp")
                      for _ in range(2)]
                if SPLIT_HALVES:
                    for ec in range(NEC):
                        for t in range(2):
                            for i in range(2):
                                half = t ^ i
                                st = st0 + i
                                nc.tensor.matmul(
                                    vp[i][:],
                                    xT[ec][H * half:H * (half + 1),
                                           P * st:P * (st + 1)],
                                    wv_sb[H * half:H * (half + 1),
                                          ec * HD:(ec + 1) * HD],
                                    start=(ec == 0 and t == 0),
                                    stop=(ec == NEC - 1 and t == 1),
                                    tile_position=(H * half, 0),
                                )
                            yield
                else:
                    for ec in range(NEC):
                        for i in range(2):
                            st = st0 + i
                            nc.tensor.matmul(
                                vp[i][:],
                                xT[ec][:, P * st:P * (st + 1)],
                                wv_sb[:, ec * HD:(ec + 1) * HD],
                                start=(ec == 0), stop=(ec == NEC - 1),
                            )
                        yield
                for i in range(2):
                    st = st0 + i
                    vslab = v_ext[:, NH * VW * st:NH * VW * (st + 1)]
                    nc.vector.tensor_copy(
                        vslab.rearrange("p (h c) -> p h c", h=NH)[:, :, 0:D],
                        vp[i].rearrange("p (h c) -> p h c", h=NH),
                    )

            def t_job(sc):
                """PE transposes: xn_all chunk sc -> xT[:][:, chunk sc]."""
                xv = xn_all.rearrange("p (k e) -> p k e", e=E)
                for ec in range(NEC):
                    pt = aux_ps.tile([P, QB], F16, tag="aux", name="pt")
                    for k in range(4):
                        nc.tensor.transpose(
                            pt[:, P * k:P * (k + 1)],
                            xv[:, 4 * sc + k, P * ec:P * (ec + 1)], ident)
                    nc.vector.tensor_copy(
                        xT[ec][:, QB * sc:QB * (sc + 1)], pt[:])
                    yield

            def proj_job(qt, ceng=None):
                """y[qt-tile] = sum_hp oT_all[hp][:, qt].T @ wp[hp]."""
                ceng = ceng or nc.vector
                if not SPLIT_HALVES:
                    ysb = y_pool.tile([P, E], F16, tag="ysb", name="ysb")
                    pj2 = [aux_ps.tile([P, 512], F32, tag="aux", name="pj")
                           for _ in range(2)]
                    for nk in range(2):
                        for hp in range(NHP):
                            nc.tensor.matmul(
                                pj2[nk][:],
                                oT_all[hp][:, P * qt:P * (qt + 1)],
                                wp_sb[hp][:, 512 * nk:512 * (nk + 1)],
                                start=(hp == 0), stop=(hp == NHP - 1),
                            )
                        yield
                    for nk in range(2):
                        if ceng is nc.scalar:
                            ceng.copy(ysb[:, 512 * nk:512 * (nk + 1)], pj2[nk][:])
                        else:
                            ceng.tensor_copy(
                                ysb[:, 512 * nk:512 * (nk + 1)], pj2[nk][:])
                    nc.sync.dma_start(out=y_d[P * qt:P * (qt + 1), :], in_=ysb)
                    return
                pj = [aux_ps.tile([P, 512], F32, tag="aux", name="pj")
                      for _ in range(2)]
                if SPLIT_HALVES:
                    for hp in range(NHP):
                        for t in range(2):
                            for nk in range(2):
                                half = t ^ nk
                                nc.tensor.matmul(
                                    pj[nk][:],
                                    oT_all[hp][H * half:H * (half + 1),
                                               P * qt:P * (qt + 1)],
                                    wp_sb[hp][H * half:H * (half + 1),
                                              512 * nk:512 * (nk + 1)],
                                    start=(hp == 0 and t == 0),
                                    stop=(hp == NHP - 1 and t == 1),
                                    tile_position=(H * half, 0),
                                )
                            yield
                else:
                    for nk in range(2):
                        for hp in range(NHP):
                            nc.tensor.matmul(
                                pj[nk][:],
                                oT_all[hp][:, P * qt:P * (qt + 1)],
                                wp_sb[hp][:, 512 * nk:512 * (nk + 1)],
                                start=(hp == 0), stop=(hp == NHP - 1),
                            )
                        yield
                ysb = y_pool.tile([P, E], F16, tag="ysb", name="ysb")
                for nk in range(2):
                    if ceng is nc.scalar:
                        ceng.copy(ysb[:, 512 * nk:512 * (nk + 1)], pj[nk][:])
                    else:
                        ceng.tensor_copy(
                            ysb[:, 512 * nk:512 * (nk + 1)], pj[nk][:])
                nc.sync.dma_start(out=y_d[P * qt:P * (qt + 1), :], in_=ysb)

            # FIFO of (chunk_tag, generator); drained as PE filler.
            jobs = []

            def chunk_jobs(c):
                # qk due at unit (c,0) start (tag 10c); v slabs of chunk c are
                # only read by the last 4 j-tiles of units (c,*), so they can
                # drain as fillers until then (tag 10c+5). Only chunk 0 is
                # PE-transposed; chunks 1-3 land in xT via DMA-transpose.
                pre = [(10 * c, t_job(c))] if c == 0 else []
                return pre + [
                    (10 * c, qk_job(c, wq_sb, qT)),
                    (10 * c, qk_job(c, wk_sb, kT)),
                    (10 * c + 5, v_job(4 * c)), (10 * c + 5, v_job(4 * c + 2))]

            def drain(n):
                k = 0
                while jobs and k < n:
                    try:
                        next(jobs[0][1])
                        k += 1
                    except StopIteration:
                        jobs.pop(0)

            def flush_tag(c):
                while jobs and jobs[0][0] <= c:
                    try:
                        next(jobs[0][1])
                    except StopIteration:
                        jobs.pop(0)

            def exhaust(gen):
                for _ in gen:
                    pass

            # ---------------- chunk 0 (no attention to overlap) ----------------
            for tag, g in chunk_jobs(0):
                if tag == 0:
                    exhaust(g)     # transposes + qT/kT
                else:
                    jobs.append((tag, g))  # v-jobs drain as early fillers

            # per-era filler drain rate (slices per j-tile): sized so the
            # filler supply lasts through the Act-heavy late eras instead of
            # bunching at unit boundaries
            RATE = (6, 2, 2, 1)

            # ---------------- attention units, pipelined with chunks 1..3 ----
            for qb in range(NQB):
                for hp in range(NHP):
                    if hp == 0:
                        if qb + 1 < NQB:
                            jobs.extend(chunk_jobs(qb + 1))
                        flush_tag(10 * qb)  # chunk qb qT/kT must be emitted
                    n_j = (qb + 1) * JPQ
                    oT2 = [oT_ps.tile([P, 512], F32, tag="oT", name="oT")
                           for _ in range(2)]
                    pend = None
                    for js in range(n_j):
                        if js == JPQ * qb and qb > 0:
                            flush_tag(10 * qb + 5)  # v slabs of chunk qb
                        cm = max(0, P * js - QB * qb)  # first visible q col
                        is_diag = js >= JPQ * qb
                        sT = sT_ps.tile([P, 1024], F32, tag="sT", name="sT")
                        for h in range(2):
                            nc.tensor.matmul(
                                sT[:, 512 * h + cm:512 * (h + 1)],
                                kT[hp][H * h:H * (h + 1), P * js:P * (js + 1)],
                                qT[hp][H * h:H * (h + 1),
                                       QB * qb + cm:QB * (qb + 1)],
                                start=True, stop=True,
                                tile_position=(H * h, 0),
                            )
                        pT = pT_pool.tile([P, 1024], F16, tag="pT", name="pT")
                        if cm == 0:
                            nc.scalar.activation(pT[:], sT[:], EXP, scale=0.125)
                        else:
                            nc.scalar.activation(
                                pT.rearrange("p (h c) -> p h c", h=2)[:, :, cm:512],
                                sT.rearrange("p (h c) -> p h c", h=2)[:, :, cm:512],
                                EXP, scale=0.125)
                        if is_diag:
                            # DVE runs fp16 SBUF multiplies at 4x — lowest
                            # latency on the exp->PV critical path
                            for h in range(2):
                                lo = 512 * h + cm
                                nc.vector.tensor_mul(
                                    pT[:, lo:lo + P], pT[:, lo:lo + P], tri[:])
                        drain(RATE[qb])
                        if pend is not None:
                            pend()

                        def make_pv(js=js, cm=cm, pT=pT, n_j=n_j):
                            def emit():
                                if SPLIT_HALVES:
                                    for t in range(2):
                                        for h in range(2):
                                            half = t ^ h
                                            hl = 2 * hp + h
                                            nc.tensor.matmul(
                                                oT2[h][0:VW, cm:512],
                                                v_ext[H * half:H * (half + 1),
                                                      NH * VW * js + VW * hl:
                                                      NH * VW * js + VW * (hl + 1)],
                                                pT[H * half:H * (half + 1),
                                                   512 * h + cm:512 * (h + 1)],
                                                start=(js == 0 and t == 0),
                                                stop=(js == n_j - 1 and t == 1),
                                                tile_position=(H * half, 0),
                                            )
                                else:
                                    for h in range(2):
                                        hl = 2 * hp + h
                                        nc.tensor.matmul(
                                            oT2[h][0:VW, cm:512],
                                            v_ext[:, NH * VW * js + VW * hl:
                                                  NH * VW * js + VW * (hl + 1)],
                                            pT[:, 512 * h + cm:512 * (h + 1)],
                                            start=(js == 0),
                                            stop=(js == n_j - 1),
                                        )
                            return emit
                        pend = make_pv()
                    pend()
                    # normalize: rows 0:64 of oT2 = numerator, row 64 = sum.
                    # One wide copy evacuates the whole oT2 bank (frees the
                    # PSUM slot for the next unit ASAP); the denominator row
                    # is then broadcast with a 1-row outer-product on the PE
                    # and reciprocal'd 64-lanes-wide on DVE.
                    osb = [None, None]
                    rb = [None, None]
                    for h in range(2):
                        osb[h] = o_pool.tile([P, 512], F16, tag="osb",
                                             name="osb")
                        nc.vector.tensor_copy(osb[h][0:VW, :], oT2[h][0:VW, :])
                        rb_ps = aux_ps.tile([P, 512], F32, tag="aux",
                                            name="rbps")
                        nc.tensor.matmul(
                            rb_ps[0:D, :], ones16[D:D + 1, :],
                            osb[h][D:D + 1, :], start=True, stop=True,
                            tile_position=(D, 0))
                        rb[h] = rb_pool.tile([D, 512], F32, tag="rb", name="rb")
                        nc.vector.reciprocal_approx_fast(rb[h][:], rb_ps[0:D, :])
                    last = (qb == NQB - 1 and hp == NHP - 1)
                    if not last:
                        for h in range(2):
                            nc.vector.tensor_mul(
                                oT_all[hp][D * h:D * (h + 1),
                                           QB * qb:QB * (qb + 1)],
                                osb[h][0:D, :], rb[h][:])
                    else:
                        # final unit: normalize per q-tile and chase each with
                        # its projection so the tail pipelines
                        for qi in range(JPQ):
                            for h in range(2):
                                nc.vector.tensor_mul(
                                    oT_all[hp][D * h:D * (h + 1),
                                               QB * qb + P * qi:
                                               QB * qb + P * (qi + 1)],
                                    osb[h][0:D, P * qi:P * (qi + 1)],
                                    rb[h][:, P * qi:P * (qi + 1)])
                            exhaust(proj_job(JPQ * qb + qi, ceng=nc.scalar))
                if qb < NQB - 1:
                    # tag beyond all guard flushes: projections drain FIFO
                    # behind chunk work, i.e. mostly in the thin qb3 era
                    jobs.extend((1000 + qb, proj_job(qt))
                                for qt in range(JPQ * qb, JPQ * (qb + 1)))
            flush_tag(9999)  # drain remaining projections

    if lower_isa:
        nc.finalize()
    return nc


_CACHED_NC = None


def _get_nc():
    global _CACHED_NC
    if _CACHED_NC is None:
        _CACHED_NC = build_core_program()
    return _CACHED_NC


def shard_inputs(x, Wq, Wk, Wv, Wp):
    in_maps = []
    x16 = [np.ascontiguousarray(x[b], dtype=np.float16) for b in range(B)]
    cident = np.eye(P, dtype=np.float16)
    ctri = np.triu(np.ones((P, P), dtype=np.float16))  # keep col >= row
    cones = np.ones((P, NST * NH), dtype=np.float16)
    for core in range(N_CORES):
        b, g = core // GROUPS, core % GROUPS
        sl = slice(HD * g, HD * (g + 1))
        in_maps.append({
            "x": x16[b],
            "wq": np.ascontiguousarray(Wq[:, sl]).astype(np.float16),
            "wk": np.ascontiguousarray(Wk[:, sl]).astype(np.float16),
            "wv": np.ascontiguousarray(Wv[:, sl]).astype(np.float16),
            "wp": np.ascontiguousarray(Wp[sl, :]).astype(np.float16),
            "cident": cident, "ctri": ctri, "cones": cones,
        })
    return in_maps


def _ensure_ntff_hook():
    """Provide antenv.axon_hooks (missing in this image) so trace=True can
    collect NTFF profiles through libaxon_pjrt's nrt-profile C ABI."""
    import types
    try:
        from antenv.axon_hooks import get_axon_ntff_profile_hook  # noqa: F401
        return
    except ImportError:
        pass
    import antenv
    mod = types.ModuleType("antenv.axon_hooks")
    mod._hook = None
    def set_axon_ntff_profile_hook(h):
        mod._hook = h
    def get_axon_ntff_profile_hook():
        return mod._hook
    mod.set_axon_ntff_profile_hook = set_axon_ntff_profile_hook
    mod.get_axon_ntff_profile_hook = get_axon_ntff_profile_hook
    sys.modules["antenv.axon_hooks"] = mod
    antenv.axon_hooks = mod
    try:
        from trn_agent_boot.trn_boot import _ntff_profile_via_ctypes
        mod._hook = _ntff_profile_via_ctypes("/opt/axon/libaxon_pjrt.so")
    except Exception as e:  # degrade: tracing skipped, run still works
        print(f"ntff hook setup failed: {e}", file=sys.stderr)


def run(inputs, trace=False, **spmd_kwargs):
    """Returns (full_output [B,S,E], BassKernelResults)."""
    from concourse.bass_utils import run_bass_kernel_spmd
    if trace:
        _ensure_ntff_hook()
    x = np.asarray(inputs["x"], dtype=np.float32)
    Wq = np.asarray(inputs["Wq"], dtype=np.float32)
    Wk = np.asarray(inputs["Wk"], dtype=np.float32)
    Wv = np.asarray(inputs["Wv"], dtype=np.float32)
    Wp = np.asarray(inputs["Wp"], dtype=np.float32)
    bp = np.asarray(inputs["bp"], dtype=np.float32)

    nc = _get_nc()
    in_maps = shard_inputs(x, Wq, Wk, Wv, Wp)
    res = run_bass_kernel_spmd(nc, in_maps, list(range(N_CORES)),
                               trace=trace, **spmd_kwargs)
    out = np.zeros((B, S, E), dtype=np.float32)
    for core in range(N_CORES):
        out[core // GROUPS] += res.results[core]["y"].astype(np.float32)
    out += bp[None, None, :]
    return out, res


def kernel(x, Wq, Wk, Wv, Wp, bp):
    out, _ = run({"x": x, "Wq": Wq, "Wk": Wk, "Wv": Wv, "Wp": Wp, "bp": bp})
    return out
